# revision 32
# baseline (speedup 1.0000x reference)
"""Causal multi-head self-attention with RoPE on 8 Trainium2 NeuronCores.

Problem: B=2, S=2048, D=2048, 16 heads x head_dim 128, causal mask, RoPE.

Sharding (data + tensor parallel per the hint): 8 cores = 2 batch rows x 4
head-groups (4 heads each). Each core computes, for its batch row and its 4
heads: Q/K/V projections, RoPE, causal softmax attention, and the partial
output projection through its head-group's slice of Wo. The host sums the 4
head-group partials per batch row (row-parallel Wo unshard).

v4.5 (l-chain elimination + causal trimming + DMA batching + tail
scheduling), on top of the bf16 + software-pipelined v3:

  - The softmax denominator no longer costs a PE pass over the weights.
    The O-accumulation matmul is reoriented to out[q,hd]: stationary = a
    128-wide q-chunk of the exp'd scores w [k,q], moving = [V | 1] (the V
    tile padded with a ones column), so column 128 of the PSUM accumulates
    sum_k w[k,q] for free. 1/l is then a per-partition scalar multiply
    (native DVE broadcast), and O is transposed back to [hd,q] for the Wo
    matmul with a cheap 128-col PE transpose.
  - Causal trimming: diagonal score tiles only compute the surviving
    column range, and O-chunk matmuls fully above the diagonal are skipped.
  - Projections for quarters >= 1 run 2 heads per pass (two passes) so
    their PSUM footprint is 2 banks; attention + everything else shares a
    6-slot PSUM ring. Quarter 0 runs 4-wide out of the shared ring.
  - DMA batching: host pre-blocks x/weights/outputs so SBUF-layout-order
    DMAs move 4 chunks at a time (the HWDGE processes descriptors serially
    at ~625ns each, so DMA count is a real cost). Weight DMAs are issued
    in first-use order; RoPE tables come as one packed cos|sin slice per
    quarter; output tiles pair two 128-row groups per DMA.
  - Quarters 1+2's Wo matmuls are deferred into the tail (quarter 3 has
    no projections left to interleave) to cover exp-latency exposure;
    their PSUM accumulators live in the then-idle projection ring, and
    their staging copies stay off the ACT (the exp stream needs it).
    Elsewhere staging copies alternate ACT/DVE, and each 128-row output
    subtile DMAs out as soon as its copy lands.
  - RoPE runs all-bf16 on the DVE; the PSUM->SBUF projection copy is on
    the ACT engine. PE-transposes for the Wo operand are emitted two
    granules after their DVE norm so the reciprocal+scale chain latency
    hides behind other PE work.
"""

import math

import numpy as np

import concourse.bass as bass
import concourse.mybir as mybir
import concourse.tile as tile
from concourse import bacc

B = 2
D = 2048
H_LOC = 4  # heads per core
HD = 128  # head dim
QW = 512  # s-quarter width (and matmul moving width)
N_CORES = 8
THETA = 10000.0
F32 = mybir.dt.float32
BF16 = mybir.dt.bfloat16


def build_program(S=2048, repeat=1):
    """Build the per-core SPMD Bass program (all 8 cores run this).

    repeat>1 re-runs the whole computation serially inside one NEFF;
    used only to measure on-device execution time via the wall-time slope.
    """
    nq = S // QW  # number of s-quarters
    dc = D // HD  # contraction chunks
    ng = dc // 4  # x/weight DMA groups (4 chunks each)
    scale = 1.0 / math.sqrt(HD)

    nc = bacc.Bacc("TRN2", target_bir_lowering=False, debug=False, num_devices=N_CORES)
    # x, blocked: [128, dc, S]; block d = x^T rows [d*128,(d+1)*128)
    xt_d = nc.dram_tensor("xt", [HD, dc, S], BF16, kind="ExternalInput").ap()
    # weights, pre-blocked to the SBUF-resident layout [128, dc*512]
    wqt_d = nc.dram_tensor("wqt", [HD, dc * QW], BF16, kind="ExternalInput").ap()
    wkt_d = nc.dram_tensor("wkt", [HD, dc * QW], BF16, kind="ExternalInput").ap()
    wvt_d = nc.dram_tensor("wvt", [HD, dc * QW], BF16, kind="ExternalInput").ap()
    wot_d = nc.dram_tensor("wot", [HD, H_LOC * D], BF16, kind="ExternalInput").ap()
    # packed per-quarter RoPE tables: quarter q at cols [q*1024,(q+1)*1024) =
    # [cos slice | sin slice]
    cs_d = nc.dram_tensor("cs", [HD, 2 * S], BF16, kind="ExternalInput").ap()
    ident_d = nc.dram_tensor("ident", [HD, HD], BF16, kind="ExternalInput").ap()
    # output, blocked like x: [128, dc, S]
    outt_d = nc.dram_tensor("outt", [HD, dc, S], BF16, kind="ExternalOutput").ap()

    with tile.TileContext(nc) as tc:
        with (
            tc.tile_pool(name="const", bufs=1) as constp,
            tc.tile_pool(name="weights", bufs=1) as wp,
            tc.tile_pool(name="ktv", bufs=1) as ktvp,
            tc.tile_pool(name="xtp", bufs=2) as xtp,
            tc.tile_pool(name="qtp", bufs=2) as qtp,
            tc.tile_pool(name="rope", bufs=2) as rtp,
            tc.tile_pool(name="wexp", bufs=20) as wep,
            tc.tile_pool(name="otT", bufs=3) as otTp,
            tc.tile_pool(name="norm", bufs=3) as nrmp,
            tc.tile_pool(name="outsb", bufs=3) as outsbp,
            tc.tile_pool(name="ps", bufs=6, space="PSUM") as psp,
        ):
            # Resident weights (bf16, loaded once at first use):
            # wq/wk/wv: [128, dc*QW] where block d holds W[d-chunk, :512].
            # wo: [128, 4*D] where block h holds Wo rows of head h.
            wk_sb = wp.tile([HD, dc * QW], BF16, tag="wk", name="wk_sb")
            wq_sb = wp.tile([HD, dc * QW], BF16, tag="wq", name="wq_sb")
            wv_sb = wp.tile([HD, dc * QW], BF16, tag="wv", name="wv_sb")
            wo_sb = wp.tile([HD, 4 * D], BF16, tag="wo", name="wo_sb")

            # per-quarter packed RoPE table slices: [cos | sin]
            cs = [
                constp.tile([HD, 2 * QW], BF16, tag=f"cs{q}", name=f"cs{q}")
                for q in range(nq)
            ]
            ident = constp.tile([HD, HD], BF16, tag="ident", name="ident_sb")
            loaded = {"w": False, "wo": False, "ident": False}
            tabs_loaded = [False] * nq

            kt = [
                ktvp.tile([HD, S], BF16, tag=f"kt{h}", name=f"kt{h}")
                for h in range(H_LOC)
            ]
            # V tiles: [k=128, head, 129] with col 128 of each head = ones
            # (the ones column makes the O-matmul accumulate the softmax
            # denominator in PSUM column 128).
            vt = [
                ktvp.tile([HD, H_LOC, HD + 1], BF16, tag=f"v{i}", name=f"v{i}")
                for i in range(S // HD)
            ]

            # pair-partner swap: +-16 within each 32-partition quadrant
            SHUF_MASK = [(i + 16) % 32 for i in range(32)]

            def rope(ps, q, out_ap):
                # out = R(pos) * ps, lane-local thanks to the host-side
                # head-dim permutation that places each RoPE pair partner 16
                # partitions away within the same 32-lane quadrant, so the
                # cross-partition move is a single DVE stream_shuffle.
                # sinpm carries the pair sign (-sin even slot, +sin odd slot).
                # The PSUM->SBUF downconvert runs on the ACT engine; the
                # rotation itself is all-bf16 on the DVE.
                rs = rtp.tile([HD, QW], BF16, tag="rs", name="rs", bufs=2)
                nc.scalar.copy(rs[:], ps[:])
                shuf = rtp.tile([HD, QW], BF16, tag="shuf", name="shuf", bufs=2)
                nc.vector.stream_shuffle(shuf[:], rs[:], SHUF_MASK)
                ta = rtp.tile([HD, QW], BF16, tag="ta", name="ta", bufs=2)
                nc.vector.tensor_mul(ta[:], rs[:], cs[q][:, 0:QW])
                tb = rtp.tile([HD, QW], BF16, tag="tb", name="tb", bufs=2)
                nc.vector.tensor_mul(tb[:], shuf[:], cs[q][:, QW : 2 * QW])
                nc.vector.tensor_add(out_ap, ta[:], tb[:])

            qts_store = {}

            def proj_granules(q, wide):
                """Projections for quarter q as (pe_cost_ns, emit_fn) list.

                wide=True: 4 heads per pass (4 PSUM slots from the shared
                ring) -- used for quarter 0 where the ring is idle and the
                DMA stream must stay ahead. wide=False: 2 heads per pass,
                two passes, 2 PSUM slots ("pp" tag).
                """
                sl = slice(q * QW, (q + 1) * QW)
                xg = [None] * ng
                pps = {}
                gran = []
                tag = "pa" if wide else "pp"
                hs_groups = [range(H_LOC)] if wide else [(0, 1), (2, 3)]

                def xsl(d):
                    return xg[d // 4][:, (d % 4) * QW : (d % 4 + 1) * QW]

                def k_chunk(d, hs, first, q=q, sl=sl):
                    def f():
                        if d == 0:
                            pps['k'] = {
                                h: psp.tile(
                                    [HD, QW], F32, tag=tag, name=f"kps{h}",
                                    bufs=6 if wide else 2,
                                )
                                for h in hs
                            }
                        if first and d % 4 == 0:
                            g = d // 4
                            x_t = xtp.tile(
                                [HD, 4 * QW], BF16, tag=f"x{g}", name=f"x{g}"
                            )
                            xg[g] = x_t
                            if q == 0 and g == 0:
                                # split + order the very first transfers so
                                # the first chunks' operands land fast
                                for xa, xb, wa, wb in (
                                    (0, 1, 0, QW),
                                    (1, 2, QW, 2 * QW),
                                    (2, 4, 2 * QW, 4 * QW),
                                ):
                                    nc.sync.dma_start(
                                        x_t[:, xa * QW : xb * QW],
                                        xt_d[:, xa:xb, sl],
                                    )
                                    if not loaded["w"]:
                                        nc.sync.dma_start(
                                            wk_sb[:, wa:wb], wkt_d[:, wa:wb]
                                        )
                            else:
                                nc.sync.dma_start(
                                    x_t[:], xt_d[:, 4 * g : 4 * (g + 1), sl]
                                )
                                if not loaded["w"]:
                                    nc.sync.dma_start(
                                        wk_sb[:, 4 * g * QW : 4 * (g + 1) * QW],
                                        wkt_d[:, 4 * g * QW : 4 * (g + 1) * QW],
                                    )
                            if g == 0 and not tabs_loaded[q]:
                                tabs_loaded[q] = True
                                nc.sync.dma_start(
                                    cs[q][:],
                                    cs_d[:, q * 2 * QW : (q + 1) * 2 * QW],
                                )
                            if g == ng - 1 and not loaded["w"]:
                                # prefetch the q-phase's first weight block so
                                # it never waits at the phase boundary
                                nc.sync.dma_start(
                                    wq_sb[:, 0 : 4 * QW], wqt_d[:, 0 : 4 * QW]
                                )
                        for h in hs:
                            nc.tensor.matmul(
                                pps['k'][h][:],
                                wk_sb[:, d * QW + h * HD : d * QW + (h + 1) * HD],
                                xsl(d),
                                start=(d == 0),
                                stop=(d == dc - 1),
                            )
                    return f

                def rope_k(h, q=q):
                    def f():
                        rope(pps['k'][h][:], q, kt[h][:, q * QW : (q + 1) * QW])
                    return f

                def q_chunk(d, hs, first, q=q):
                    def f():
                        if d == 0:
                            pps['q'] = {
                                h: psp.tile(
                                    [HD, QW], F32, tag=tag, name=f"qps{h}",
                                    bufs=6 if wide else 2,
                                )
                                for h in hs
                            }
                        if first and d % 4 == 0 and not loaded["w"]:
                            g = d // 4
                            if g > 0:  # block 0 was prefetched in the k-phase
                                nc.sync.dma_start(
                                    wq_sb[:, 4 * g * QW : 4 * (g + 1) * QW],
                                    wqt_d[:, 4 * g * QW : 4 * (g + 1) * QW],
                                )
                            if g == ng - 1:
                                nc.sync.dma_start(
                                    wv_sb[:, 0 : 4 * QW], wvt_d[:, 0 : 4 * QW]
                                )
                        for h in hs:
                            nc.tensor.matmul(
                                pps['q'][h][:],
                                wq_sb[:, d * QW + h * HD : d * QW + (h + 1) * HD],
                                xsl(d),
                                start=(d == 0),
                                stop=(d == dc - 1),
                            )
                    return f

                def rope_q(h, q=q):
                    def f():
                        q_sb = qtp.tile([HD, QW], BF16, tag=f"qt{h}", name=f"qt{h}")
                        rope(pps['q'][h][:], q, q_sb[:])
                        qts_store[q][h] = q_sb
                    return f

                def v_chunk(d, sts, first, q=q):
                    def f():
                        if d == 0:
                            pps['v'] = {
                                st: psp.tile(
                                    [HD, H_LOC, HD], F32, tag=tag, name=f"vps{st}",
                                    bufs=6 if wide else 2,
                                )
                                for st in sts
                            }
                        if first and d % 4 == 0 and not loaded["w"]:
                            g = d // 4
                            if g > 0:  # block 0 was prefetched in the q-phase
                                nc.sync.dma_start(
                                    wv_sb[:, 4 * g * QW : 4 * (g + 1) * QW],
                                    wvt_d[:, 4 * g * QW : 4 * (g + 1) * QW],
                                )
                            if g == ng - 1:
                                loaded["w"] = True
                        for st in sts:
                            nc.tensor.matmul(
                                pps['v'][st][:, :, :],
                                xg[d // 4][
                                    :, (d % 4) * QW + st * HD
                                    : (d % 4) * QW + (st + 1) * HD
                                ],
                                wv_sb[:, d * QW : (d + 1) * QW],
                                start=(d == 0),
                                stop=(d == dc - 1),
                            )
                    return f

                def v_copy(st, q=q):
                    def f():
                        # scatter the 4 heads' V into the ones-padded layout
                        v = vt[q * 4 + st]
                        nc.vector.tensor_copy(v[:, :, 0:HD], pps['v'][st][:, :, :])
                        nc.vector.memset(v[:, :, HD : HD + 1], 1.0)
                    return f

                qts_store[q] = [None] * H_LOC
                mm = 853 if wide else 427
                for gi, hs in enumerate(hs_groups):
                    for d in range(dc):
                        gran.append((mm, k_chunk(d, hs, first=(gi == 0))))
                    for h in hs:
                        gran.append((30, rope_k(h)))
                for gi, hs in enumerate(hs_groups):
                    for d in range(dc):
                        gran.append((mm, q_chunk(d, hs, first=(gi == 0))))
                    for h in hs:
                        gran.append((30, rope_q(h)))
                for gi, sts in enumerate(hs_groups):
                    for d in range(dc):
                        gran.append((mm, v_chunk(d, sts, first=(gi == 0))))
                    for st in sts:
                        gran.append((20, v_copy(st)))
                return gran

            def attn_granules(q):
                """Attention for quarter q as (pe_cost_ns, emit_fn) list."""
                nk = (q + 1) * 4
                state = {'wts': {}, 'otT': [None] * H_LOC}
                gran = []

                def score_g(h, ki, q=q, nk=nk):
                    # scores^T tile [k=128, q-range] -> exp -> w (bf16 SBUF);
                    # diagonal tiles only compute surviving columns.
                    diag = ki >= 4 * q
                    off = (ki - 4 * q) * HD if diag else 0
                    def f():
                        s_ps = psp.tile([HD, QW], F32, tag="pa", name="s_ps")
                        nc.tensor.matmul(
                            s_ps[:, off:],
                            kt[h][:, ki * HD : (ki + 1) * HD],
                            qts_store[q][h][:, off:],
                            start=True,
                            stop=True,
                        )
                        w_t = wep.tile([HD, QW], BF16, tag="wexp", name="w_t")
                        nc.scalar.activation(
                            w_t[:, off:],
                            s_ps[:, off:],
                            mybir.ActivationFunctionType.Exp,
                            scale=scale,
                        )
                        if diag:
                            # only the 128-col chunk straddling the diagonal
                            # is partially masked; columns beyond it are
                            # fully valid. Masking just that chunk keeps og
                            # matmuls of later chunks off this Pool op's
                            # dependency chain (and quarters Pool work).
                            nc.gpsimd.affine_select(
                                out=w_t[:, off : off + HD],
                                in_=w_t[:, off : off + HD],
                                compare_op=mybir.AluOpType.is_ge,
                                fill=0.0,
                                base=q * QW - ki * HD + off,
                                pattern=[[1, HD]],
                                channel_multiplier=-1,
                            )
                        state['wts'][h, ki] = w_t
                    return f

                def og(h, c, q=q, nk=nk):
                    # O'[q-chunk, 0:128] = sum_k V^T w ; O'[:, 128] = sum_k w
                    # (the l column). Stationary = w q-chunk, moving = [V|1].
                    cg = 4 * q + c
                    cnt = min(nk, cg + 1)
                    def f():
                        if loaded["ident"] is False:
                            loaded["ident"] = True
                            nc.sync.dma_start(ident[:], ident_d[:])
                            nc.sync.dma_start(wo_sb[:], wot_d[:])
                        ot = psp.tile([HD, HD + 1], F32, tag="pa", name="ot")
                        state['ot', c] = ot
                        for ki in range(cnt):
                            nc.tensor.matmul(
                                ot[:],
                                state['wts'][h, ki][:, c * HD : (c + 1) * HD],
                                vt[ki][:, h : h + 1, :],
                                start=(ki == 0),
                                stop=(ki == cnt - 1),
                            )
                    return f

                def norm_dve(h, c, q=q):
                    # 1/l is a per-partition scalar in the [q,hd] layout:
                    # native free-axis broadcast on the DVE, no
                    # partition_broadcast needed.
                    def f():
                        ot = state['ot', c]
                        rc = nrmp.tile([HD, 1], F32, tag="rc", name="rc")
                        nc.vector.reciprocal(rc[:], ot[:, HD : HD + 1])
                        o_sb = nrmp.tile([HD, HD], BF16, tag="osb", name="o_sb")
                        nc.vector.tensor_scalar_mul(o_sb[:], ot[:, 0:HD], rc[:])
                        state['osb', c] = o_sb
                    return f

                def norm_tr(h, c, q=q):
                    # transpose O[q,hd] -> O^T[hd,q] for the Wo matmul
                    def f():
                        otT_ps = psp.tile([HD, HD], BF16, tag="pa", name="otT_ps")
                        nc.tensor.transpose(otT_ps[:], state['osb', c][:], ident[:])
                        if c == 0:
                            state['otT'][h] = otTp.tile(
                                [HD, QW], BF16, tag=f"otT{h}", name=f"otT{h}"
                            )
                        nc.vector.tensor_copy(
                            state['otT'][h][:, c * HD : (c + 1) * HD], otT_ps[:]
                        )
                    return f

                def emit_og(h):
                    # stagger: the PE transpose for chunk c is emitted two
                    # granules after its DVE norm so the reciprocal+scale
                    # chain latency hides behind other PE work.
                    for c in range(H_LOC):
                        cnt = min(nk, 4 * q + c + 1)
                        gran.append((54 * cnt, og(h, c)))
                        gran.append((1, norm_dve(h, c)))
                        if c >= 1:
                            gran.append((60, norm_tr(h, c - 1)))
                    gran.append((60, norm_tr(h, H_LOC - 1)))

                def emit_scores(h):
                    for ki in range(nk):
                        diag = ki >= 4 * q
                        w = QW - ((ki - 4 * q) * HD if diag else 0)
                        gran.append((int(213 * w / QW), score_g(h, ki)))

                if q == 0:
                    # quarter 0 per-head work is tiny (4 score tiles); all
                    # heads' scores are returned as a separate list that the
                    # caller merges into the prologue's V-phase (the ACT is
                    # idle there), so the og matmuls never wait on exps.
                    sgran = []
                    gran_sv = gran
                    gran = sgran
                    for h in range(H_LOC):
                        emit_scores(h)
                    gran = gran_sv
                    for h in range(H_LOC):
                        emit_og(h)
                    return gran, sgran, state
                else:
                    # lag og(0) two score-granules past its last exp dep so
                    # the ACT latency never stalls the PE
                    for h in range(H_LOC):
                        for ki in range(nk - 2):
                            w = QW - ((ki - 4 * q) * HD if ki >= 4 * q else 0)
                            gran.append((int(213 * w / QW), score_g(h, ki)))
                        cnt = min(nk, 4 * q + 1)
                        gran.append((54 * cnt, og(h, 0)))
                        gran.append((1, norm_dve(h, 0)))
                        for ki in range(nk - 2, nk):
                            w = QW - (ki - 4 * q) * HD
                            gran.append((int(213 * w / QW), score_g(h, ki)))
                        ogw = 54
                        for c in range(1, H_LOC):
                            cnt = min(nk, 4 * q + c + 1)
                            gran.append((ogw * cnt, og(h, c)))
                            gran.append((1, norm_dve(h, c)))
                            if c >= 2:
                                gran.append((60, norm_tr(h, c - 2)))
                        gran.append((60, norm_tr(h, H_LOC - 2)))
                        gran.append((60, norm_tr(h, H_LOC - 1)))
                return gran, state

            def wo_granules(q, attn_state, copy_eng="mixed", ps_tag="pa"):
                """Wo for quarter q: out^T[d,q] += Wo_h^T O_h^T, 2 d-subtiles
                per PSUM group (2 shared-ring slots). Output staging copies
                alternate ACT/DVE (copy_eng="mixed") or stay off the ACT
                ("dve") where the exp stream saturates it; the two subtiles
                go out in one DMA."""
                sl = slice(q * QW, (q + 1) * QW)
                gran = []
                wst = {}

                def wo_g(g, h, q=q):
                    def f():
                        if h == 0:
                            wst[g] = [
                                psp.tile(
                                    [HD, 1, QW], F32, tag=ps_tag, name=f"ops{dt}",
                                    bufs=6 if ps_tag == "pa" else 2,
                                )
                                for dt in range(2)
                            ]
                        for dt in range(2):
                            nc.tensor.matmul(
                                wst[g][dt][:, :, :],
                                wo_sb[
                                    :,
                                    h * D + g * 2 * HD + dt * HD
                                    : h * D + g * 2 * HD + (dt + 1) * HD,
                                ],
                                attn_state['otT'][h][:],
                                start=(h == 0),
                                stop=(h == H_LOC - 1),
                            )
                    return f

                def wo_out(g, q=q, sl=sl):
                    def f():
                        # per-subtile DMAs so the first can start while the
                        # second subtile is still being staged
                        o2 = outsbp.tile([HD, 2, QW], BF16, tag="osb", name="o2")
                        if copy_eng == "mixed":
                            nc.scalar.copy(o2[:, 0:1, :], wst[g][0][:, :, :])
                        else:
                            nc.vector.tensor_copy(o2[:, 0:1, :], wst[g][0][:, :, :])
                        eng0 = nc.scalar if q == 3 else nc.sync
                        eng0.dma_start(
                            outt_d[:, 2 * g : 2 * g + 1, sl], o2[:, 0:1, :]
                        )
                        nc.vector.tensor_copy(o2[:, 1:2, :], wst[g][1][:, :, :])
                        nc.sync.dma_start(
                            outt_d[:, 2 * g + 1 : 2 * g + 2, sl], o2[:, 1:2, :]
                        )
                    return f

                for g in range(8):
                    for h in range(H_LOC):
                        gran.append((427, wo_g(g, h)))
                    gran.append((40, wo_out(g)))
                return gran

            def merge(a, b):
                """Interleave two granule lists by cumulative-cost ratio."""
                ca = sum(c for c, _ in a) or 1
                cb = sum(c for c, _ in b) or 1
                ia = ib = 0
                ra = rb = 0.0
                while ia < len(a) or ib < len(b):
                    if ib >= len(b) or (
                        ia < len(a)
                        and (ra + a[ia][0]) / ca <= (rb + b[ib][0]) / cb
                    ):
                        ra += a[ia][0]
                        a[ia][1]()
                        ia += 1
                    else:
                        rb += b[ib][0]
                        b[ib][1]()
                        ib += 1

            # rep 0 prologue: quarter-0 projections run alone (wide,
            # DMA-paced) except the V-phase, which hosts quarter-0's score
            # matmuls (the ACT is idle there). Later reps' quarter-0
            # projections and scores merge into the previous rep's tail.
            og0, sc0, st0 = attn_granules(0)
            proj0 = proj_granules(0, wide=True)
            for _, f in proj0[: 2 * (dc + H_LOC)]:
                f()
            merge(sc0, proj0[2 * (dc + H_LOC) :])
            for r in range(repeat):
                att = {}
                merge(og0 + wo_granules(0, st0), proj_granules(1, wide=False))
                att[1], st1 = attn_granules(1)
                merge(att[1], proj_granules(2, wide=False))
                att[2], st2 = attn_granules(2)
                merge(att[2], proj_granules(3, wide=False))
                # tail: quarter 3 has no projections to interleave, so the
                # deferred Wo work of quarters 1+2 fills the exp-latency gaps
                # (staging copies stay off the ACT: the exp stream needs it).
                # Last rep: the proj PSUM ring is idle in the tail, so the
                # deferred Wo accumulators live there instead of stealing
                # shared-ring slots from quarter-3 attention. Earlier reps
                # merge the next rep's projections into the tail, so the
                # proj ring is NOT free there (using it would deadlock the
                # in-order PE on slot waits).
                wtag = "pp" if r + 1 >= repeat else "pa"
                wo12 = wo_granules(1, st1, copy_eng="dve", ps_tag=wtag) + \
                    wo_granules(2, st2, copy_eng="dve", ps_tag=wtag)
                att[3], st3 = attn_granules(3)
                if r + 1 < repeat:
                    merge(att[3] + wo12, proj_granules(0, wide=False))
                    og0, sc0, st0 = attn_granules(0)
                    # the next rep's quarter-0 scores ride along with this
                    # rep's final Wo block (exp-heavy meets PE-heavy)
                    merge(wo_granules(3, st3), sc0)
                else:
                    merge(att[3], wo12)
                    for _, f in wo_granules(3, st3):
                        f()
    nc.compile()
    return nc


def prep_inputs(x, token_positions, Wq, Wk, Wv, Wo):
    """Shard + lay out the full inputs into 8 per-core input maps."""
    import ml_dtypes

    bf16 = ml_dtypes.bfloat16
    S = x.shape[1]
    dc = D // HD
    x = np.asarray(x, np.float32)
    pos = np.asarray(token_positions).astype(np.float32)
    k = np.arange(HD // 2, dtype=np.float32)
    inv_freq = (1.0 / (THETA ** (2.0 * k / HD))).astype(np.float32)
    freqs = pos[:, None] * inv_freq[None, :]  # [S, 64]
    cos = np.cos(freqs).T.astype(np.float32)  # [64, S]
    sin = np.sin(freqs).T.astype(np.float32)
    # head-dim permutation chosen so each RoPE pair partner sits +-16
    # partitions away within the same 32-partition quadrant (enables the
    # on-device stream_shuffle). Partition n holds:
    #   g, r = divmod(n, 32); j = 16*g + (r % 16)   (frequency index)
    #   original dim 2j   if r < 16 ("even" slot, rotates with -sin)
    #   original dim 2j+1 otherwise ("odd" slot, rotates with +sin)
    n = np.arange(HD)
    g, r = n // 32, n % 32
    j = 16 * g + (r % 16)
    odd = (r >= 16).astype(np.int64)
    perm = 2 * j + odd
    cos2 = cos[j].astype(np.float32)  # [128, S]
    sinpm = np.where(odd[:, None], sin[j], -sin[j]).astype(np.float32)
    # packed per-quarter [cos | sin] slices: [128, 2*S]
    nq = S // QW
    cs = np.concatenate(
        [
            np.concatenate(
                [cos2[:, q * QW : (q + 1) * QW], sinpm[:, q * QW : (q + 1) * QW]],
                axis=1,
            )
            for q in range(nq)
        ],
        axis=1,
    ).astype(bf16)
    ident = np.eye(HD, dtype=bf16)

    def blockT(a):  # [D, W] -> [128, (D//128)*W] chunk-blocked
        Dd, W = a.shape
        return np.ascontiguousarray(
            a.reshape(Dd // HD, HD, W).transpose(1, 0, 2).reshape(HD, -1)
        )

    xts = [blockT(x[b].T).astype(bf16) for b in range(B)]  # [128, dc*S]

    in_maps = []
    for c in range(N_CORES):
        b, hg = c // 4, c % 4
        rows = slice(hg * H_LOC * HD, (hg + 1) * H_LOC * HD)

        def permW(W):
            Wg = np.asarray(W, np.float32)[rows]  # [512, D]
            Wg = Wg.reshape(H_LOC, HD, D)[:, perm, :].reshape(H_LOC * HD, D)
            return blockT(Wg.T).astype(bf16)  # [128, dc*512]

        in_maps.append(
            {
                "xt": xts[b],
                "wqt": permW(Wq),
                "wkt": permW(Wk),
                "wvt": blockT(np.asarray(Wv, np.float32)[rows].T).astype(bf16),
                "wot": blockT(
                    np.ascontiguousarray(np.asarray(Wo, np.float32)[:, rows].T)
                ).astype(bf16),
                "cs": cs,
                "ident": ident,
            }
        )
    return in_maps


def combine_outputs(outts):
    """outts: list of 8 per-core blocked outT [128, dc, S] partials ->
    full [B, S, D]."""
    dc = D // HD
    outs = []
    for b in range(B):
        acc = sum(
            np.asarray(o, np.float32).reshape(HD, dc, -1)
            for o in outts[b * 4 : (b + 1) * 4]
        )
        # [128, dc, S] -> [D, S] -> [S, D]
        outs.append(acc.transpose(1, 0, 2).reshape(D, -1).T.astype(np.float32))
    return np.stack(outs)


_NC = None


def _get_nc():
    global _NC
    if _NC is None:
        _NC = build_program()
    return _NC


def kernel(x, token_positions, Wq, Wk, Wv, Wo):
    from concourse.bass_utils import run_bass_kernel_spmd

    nc = _get_nc()
    in_maps = prep_inputs(x, token_positions, Wq, Wk, Wv, Wo)
    res = run_bass_kernel_spmd(nc, in_maps, core_ids=list(range(N_CORES)))
    return combine_outputs([r["outt"] for r in res.results])
